# revision 61
# baseline (speedup 1.0000x reference)
"""Trainium2 Bass kernel for nn_Attention_45578192945380.

Full (unsharded) inputs -> full output. Sharding: core c handles batch b=c//2
and head group g=c%2 (heads 4g..4g+4). Zero cross-core communication; the two
cores sharing a batch produce partial out-projections that are summed on host.

Key structure (v2, rewritten from the 229us baseline):
  - softmax_j(s_ij + B*bias_i + B*bias_j) == softmax_j(s_ij + B*bias_j); the
    remaining e^{B*bias_j} factor is folded into the AV weight columns
    (v_j * e^{b_j} and an e^{b_j} "denominator" block), making the exp input
    bias-free so one ACTIVATE can span any PSUM layout.
  - scores computed transposed (sT[j,i] = k_j . q_i); AV lhsT = [v*eb | eb]
    with the eb block 64 wide, so AV rows 64:127 are the softmax denominator
    replicated 64x -> reciprocal+mult directly from PSUM, no broadcasts.
  - xn transpose done by the DMA XBAR (dma_start_transpose), not the PE.
  - attention processes head PAIRS with row-tiled concurrent K=64 dots
    matmuls (tile_position (0,0)/(64,0)) -> ~2x dots throughput.
  - exp is split across engines: ScalarE table exp for head0 (+ fused
    both-head reads late in each block), and a Schraudolph-style fast exp on
    VectorE for head1 (affine in log2 domain -> int16 round -> bits are bf16).
    Calibrated vs HW: convert is round-to-nearest; B tuned for zero-mean
    relative error (max ~4%, cancels in the softmax average).
  - out-projection streams per 512-col i-block as soon as both head-pairs
    finished that block; results DMA PSUM->DRAM in fp32, host sums partials.
"""

import os
import sys
from contextlib import ExitStack

import numpy as np

for _p in ("/opt/trn_rl_repo", "/root/.axon_site/_ro/trn_rl_repo"):
    if os.path.isdir(_p) and _p not in sys.path:
        sys.path.insert(0, _p)

import ml_dtypes

import concourse.bass as bass
import concourse.bacc as bacc
import concourse.tile as tile
from concourse import mybir
from concourse.bass_utils import run_bass_kernel_spmd

F32 = mybir.dt.float32
BF16 = mybir.dt.bfloat16
I16 = mybir.dt.int16
AF = mybir.ActivationFunctionType
OP = mybir.AluOpType
BFNP = ml_dtypes.bfloat16

B, N, DIM = 4, 2048, 512
HEADS, DH = 8, 64
EPS = 1e-5
NT = N // 128          # 16 n-chunks of 128
DC = DIM // 128        # 4 d-chunks
SCALE = DH ** -0.5     # 0.125
NCORES = 8

# Schraudolph fast-exp in bf16 bits: bf16_bits(e^s) ~= rint(A_SCH*s + B_SCH)
# (A_SCH folds the 1/sqrt(dh) score scale; B_SCH calibrated for zero-mean
# relative error under the HW's round-to-nearest fp32->int16 convert.)
A_SCH = (128.0 / float(np.log(2.0))) * SCALE
B_SCH = 16248.62
DVE_J = 12             # per (pair, iq): J < DVE_J -> head1 exp on VectorE
DEBUG_TAPS = False     # emit intermediate-tensor DMAs for debugging


def _emit(tc: tile.TileContext, ctx: ExitStack, aps: dict, affine: bool):
    nc = tc.nc

    const = ctx.enter_context(tc.tile_pool(name="const", bufs=1))
    big = ctx.enter_context(tc.tile_pool(name="big", bufs=1))

    # ---------------- input DMAs (coalesced) ----------------
    # x in 4 group-issues on the sync HWDGE queue; weights/pb on gpsimd SWDGE.
    # x spread across all three DMA rings: each ring drains its DMAs
    # serially, so putting everything on one ring serializes the prologue
    x_g = [big.tile([128, 4, DIM], F32, name=f"x{g}") for g in range(4)]
    xr = aps["x"].rearrange("(t p) d -> p t d", p=128)
    nc.sync.dma_start(out=x_g[0][:, 0:2, :], in_=xr[:, 0:2, :])
    nc.sync.dma_start(out=x_g[0][:, 2:4, :], in_=xr[:, 2:4, :])
    nc.scalar.dma_start(out=x_g[1], in_=xr[:, 4:8, :])
    nc.gpsimd.dma_start(out=x_g[2], in_=xr[:, 8:12, :])
    nc.gpsimd.dma_start(out=x_g[3], in_=xr[:, 12:16, :])

    wq_sb = const.tile([128, DC, 256], BF16)
    wk_sb = const.tile([128, DC, 256], BF16)
    wv_sb = const.tile([128, DC, 256], BF16)
    wo_sb = const.tile([128, 2, 512], BF16)
    nc.gpsimd.dma_start(out=wq_sb, in_=aps["wq"].rearrange("(c p) m -> p c m", p=128))
    nc.gpsimd.dma_start(out=wk_sb, in_=aps["wk"].rearrange("(c p) m -> p c m", p=128))
    nc.gpsimd.dma_start(out=wv_sb, in_=aps["wv"].rearrange("(c p) m -> p c m", p=128))
    nc.gpsimd.dma_start(out=wo_sb, in_=aps["wo"].rearrange("(c p) m -> p c m", p=128))
    pb_sb = const.tile([128, NT], F32)
    nc.gpsimd.dma_start(out=pb_sb[:, :], in_=aps["pb"].rearrange("(t p) -> p t", p=128))

    gam_bc = bet_bc = None
    if affine:
        gam_bc = const.tile([128, DIM], F32)
        bet_bc = const.tile([128, DIM], F32)
        ga, be = aps["gam"], aps["bet"]
        nc.sync.dma_start(
            out=gam_bc[:, :],
            in_=bass.AP(tensor=ga.tensor, offset=ga.offset, ap=[[0, 128]] + list(ga.ap)),
        )
        nc.sync.dma_start(
            out=bet_bc[:, :],
            in_=bass.AP(tensor=be.tensor, offset=be.offset, ap=[[0, 128]] + list(be.ap)),
        )

    # ---------------- persistent SBUF ----------------
    xnT_g = [big.tile([128, DC, 512], BF16, name=f"xnT{g}") for g in range(4)]
    qT_t = [[big.tile([128, 512], BF16, name=f"qT{cc}{p}") for p in range(4)]
            for cc in range(2)]
    kT_t = [[big.tile([128, 512], BF16, name=f"kT{cc}{p}") for p in range(4)]
            for cc in range(2)]
    v_g = [big.tile([128, 4, 4, 128], BF16, name=f"v{g}") for g in range(4)]
    aoT = [big.tile([128, N], BF16, name=f"aoT{cc}") for cc in range(2)]
    stats = const.tile([128, NT, 2], F32)
    lnv = const.tile([128, NT], F32)
    rstd = const.tile([128, NT], F32)
    negmr = const.tile([128, NT], F32)
    pbe = const.tile([128, NT], F32)
    eps_sb = const.tile([128, 1], F32)
    nc.vector.memset(eps_sb, EPS)
    zero_sb = const.tile([128, 1], F32)
    nc.vector.memset(zero_sb, 0.0)
    ones256 = const.tile([128, 256], BF16)
    nc.gpsimd.memset(ones256, 1.0)

    # pbe = exp(beta*pb) (pb premultiplied by beta on host)
    nc.scalar.activation(out=pbe, in_=pb_sb, func=AF.Exp, bias=zero_sb[:, :])

    # ---------------- PSUM pools ----------------
    # dots: 4 single-bank tiles; av: double-buffered 2-bank accumulators.
    # qk pieces / v chunks / out-proj borrow dots slots.
    dots_pool = ctx.enter_context(tc.tile_pool(name="dots", bufs=6, space="PSUM"))
    av_pool = ctx.enter_context(tc.tile_pool(name="av", bufs=1, space="PSUM"))

    ph1 = ctx.enter_context(tc.tile_pool(name="ph1", bufs=4))
    epool = ctx.enter_context(tc.tile_pool(name="epool", bufs=8))
    rpool = ctx.enter_context(tc.tile_pool(name="rpool", bufs=2))
    spool = ctx.enter_context(tc.tile_pool(name="spool", bufs=4))

    # debug.py relies on this layout note: v_g even heads [v*eb | eb],
    # odd heads [eb | v*eb].

    # ---------------- LN stats ----------------
    def finalize_stats(sl):
        nc.scalar.activation(out=lnv[:, sl], in_=stats[:, sl, 1], func=AF.Ln, bias=eps_sb[:, :])
        nc.scalar.activation(out=rstd[:, sl], in_=lnv[:, sl], func=AF.Exp, bias=zero_sb[:, :], scale=-0.5)
        nc.vector.tensor_scalar(out=stats[:, sl, 0], in0=stats[:, sl, 0], scalar1=-1.0, scalar2=None, op0=OP.mult)
        nc.vector.tensor_tensor_scan(out=negmr[:, sl], data0=stats[:, sl, 0], data1=rstd[:, sl],
                                     initial=0.0, op0=OP.bypass, op1=OP.mult)

    # ---------------- LN apply + XBAR transpose + V + QK ----------------
    def qk_piece(w_sb, dst, cc, p, evac_eng):
        ps = dots_pool.tile([128, 512], F32, tag="d", name=f"qk_{id(dst)}_{cc}_{p}")
        for dc in range(DC):
            nc.tensor.matmul(
                ps, w_sb[:, dc, cc * 128:(cc + 1) * 128],
                xnT_g[p][:, dc, :],
                start=(dc == 0), stop=(dc == DC - 1),
            )
        if evac_eng == "scalar":
            nc.scalar.copy(out=dst, in_=ps)
        else:
            nc.vector.tensor_copy(out=dst, in_=ps)

    # LN stats for all chunks first (gated only by the x DMAs)
    for g in range(4):
        for tt in range(4):
            t = 4 * g + tt
            st6 = ph1.tile([128, 6], F32, tag="bnst")
            nc.vector.bn_stats(out=st6, in_=x_g[g][:, tt, :])
            nc.vector.bn_aggr(out=stats[:, t, :], in_=st6)
        finalize_stats(slice(4 * g, 4 * g + 4))

    def prologue_group(g):
        """LN + transpose + QK(cc0) + V for one 4-chunk group."""
        for tt in range(4):
            t = 4 * g + tt
            if affine:
                xh = ph1.tile([128, DIM], F32, tag="xh")
                nc.vector.tensor_scalar(
                    out=xh, in0=x_g[g][:, tt, :],
                    scalar1=rstd[:, t:t + 1], scalar2=negmr[:, t:t + 1],
                    op0=OP.mult, op1=OP.add,
                )
                xg = ph1.tile([128, DIM], F32, tag="xg")
                nc.vector.tensor_tensor(out=xg, in0=xh, in1=gam_bc[:, :], op=OP.mult)
                xn_t = ph1.tile([128, DIM], BF16, tag="xn")
                nc.vector.tensor_tensor(out=xn_t, in0=xg, in1=bet_bc[:, :], op=OP.add)
            else:
                xn_t = ph1.tile([128, DIM], BF16, tag="xn")
                nc.vector.tensor_scalar(
                    out=xn_t, in0=x_g[g][:, tt, :],
                    scalar1=rstd[:, t:t + 1], scalar2=negmr[:, t:t + 1],
                    op0=OP.mult, op1=OP.add,
                )
            # XBAR transpose: out[p, dc, j] = xn[j, dc*128+p]
            teng = nc.sync if tt % 2 == 0 else nc.scalar
            teng.dma_start_transpose(
                out=xnT_g[g][:, :, tt * 128:(tt + 1) * 128], in_=xn_t)
        # QK pieces (cc0 only; cc1 is injected into later attention blocks)
        qk_piece(wq_sb, qT_t[0][g], 0, g, "vector")
        qk_piece(wk_sb, kT_t[0][g], 0, g, "vector")
        # V chunks: even heads [v*eb | eb], odd heads [eb | v*eb] — each
        # head's AV numerator lands on the partition half its aoT rows live
        # on, the denominator on the other (DVE lanes are partition-locked;
        # the reciprocal is DMA-moved across at finalize).
        for tt in range(4):
            t = 4 * g + tt
            ps = dots_pool.tile([128, 512], F32, tag="d", name=f"vps_{t}")
            for dc in range(DC):
                nc.tensor.matmul(
                    ps[:, 0:256], xnT_g[g][:, dc, tt * 128:(tt + 1) * 128],
                    wv_sb[:, dc, :],
                    start=(dc == 0), stop=(dc == DC - 1),
                )
            va = v_g[g][:, tt, :, :]
            v_dst = bass.AP(tensor=va.tensor, offset=va.offset,
                            ap=[list(va.ap[0]), [256, 2], [192, 2], [1, 64]])
            vin = ps[:, 0:256].rearrange("p (a b x) -> p a b x", a=2, b=2)
            nc.scalar.activation(out=v_dst, in_=vin, func=AF.Copy,
                                 bias=0.0, scale=pbe[:, t:t + 1])
            e_dst = bass.AP(tensor=va.tensor, offset=va.offset + 64,
                            ap=[list(va.ap[0]), [256, 2], [64, 2], [1, 64]])
            nc.gpsimd.tensor_scalar(
                out=e_dst,
                in0=ones256.rearrange("p (a b x) -> p a b x", a=2, b=2),
                scalar1=pbe[:, t:t + 1], scalar2=None, op0=OP.mult,
            )

    prologue_group(0)

    # ---------------- attention: head pairs ----------------
    def outproj_mc(p, mc):
        po = dots_pool.tile([128, 512], F32, tag="d", name=f"po_{p}_{mc}")
        nc.tensor.matmul(po, wo_sb[:, 0, mc * 128:(mc + 1) * 128],
                         aoT[0][:, p * 512:(p + 1) * 512], start=True, stop=False)
        nc.tensor.matmul(po, wo_sb[:, 1, mc * 128:(mc + 1) * 128],
                         aoT[1][:, p * 512:(p + 1) * 512], start=False, stop=True)
        st = spool.tile([128, 512], BF16, tag="st")
        if mc % 2 == 0:
            nc.scalar.copy(out=st, in_=po)
        else:
            nc.vector.tensor_copy(out=st, in_=po)
        nc.gpsimd.dma_start(
            out=aps["out"][mc * 128:(mc + 1) * 128, p * 512:(p + 1) * 512],
            in_=st,
        )

    pending_op = []  # out-proj i-blocks deferred into the next block's J-loop
    for cc in range(2):
        h0, h1 = 2 * cc, 2 * cc + 1
        for iq in range(4):
            # remaining prologue groups pipeline into the first block;
            # cc=1 QK pieces are injected near the end of each cc=0 block
            hooks = {}
            if cc == 0 and iq == 0:
                hooks = {2: lambda: prologue_group(1),
                         5: lambda: prologue_group(2),
                         9: lambda: prologue_group(3)}
            if cc == 0:
                hooks[13] = lambda p=iq: qk_piece(wq_sb, qT_t[1][p], 1, p, "scalar")
                hooks[14] = lambda p=iq: qk_piece(wk_sb, kT_t[1][p], 1, p, "scalar")
            av = av_pool.tile([128, 2, 512], F32, tag="av", name=f"av_{cc}_{iq}")
            pend = []

            def emit_av(Jp, e0p, e1p):
                g, tt = Jp // 4, Jp % 4
                nc.tensor.matmul(av[:, 0, :], v_g[g][:, tt, h0, :], e0p,
                                 start=(Jp == 0), stop=(Jp == NT - 1))
                nc.tensor.matmul(av[:, 1, :], v_g[g][:, tt, h1, :], e1p,
                                 start=(Jp == 0), stop=(Jp == NT - 1))

            for J in range(NT):
                if J in hooks:
                    hooks[J]()
                d0 = dots_pool.tile([128, 512], F32, tag="d", name=f"d0_{cc}_{iq}_{J}")
                d1 = dots_pool.tile([128, 512], F32, tag="d", name=f"d1_{cc}_{iq}_{J}")
                kp = kT_t[cc][J // 4]
                ksl = slice((J % 4) * 128, (J % 4) * 128 + 128)
                qp = qT_t[cc][iq]
                nc.tensor.matmul(d0, kp[0:64, ksl], qp[0:64, :],
                                 start=True, stop=True)
                nc.tensor.matmul(d1, kp[64:128, ksl], qp[64:128, :],
                                 start=True, stop=True)
                # separate eT tiles per head: ACT and DVE each own one, so the
                # two exp engines never serialize on a shared writer tile
                e0 = epool.tile([128, 512], BF16, tag="e0")
                e1 = epool.tile([128, 512], BF16, tag="e1")
                nc.scalar.activation(out=e0, in_=d0,
                                     func=AF.Exp, bias=zero_sb[:, :], scale=SCALE)
                if J < DVE_J:
                    nc.vector.tensor_scalar(
                        out=e1.bitcast(I16), in0=d1,
                        scalar1=A_SCH, scalar2=B_SCH, op0=OP.mult, op1=OP.add,
                    )
                else:
                    nc.scalar.activation(out=e1, in_=d1,
                                         func=AF.Exp, bias=zero_sb[:, :], scale=SCALE)
                if len(pend) >= 2:
                    emit_av(*pend.pop(0))
                pend.append((J, e0, e1))
                # spread a deferred out-proj column block into this J-stream
                if pending_op and J in (6, 8, 10, 12):
                    outproj_mc(*pending_op.pop(0))
            for item in pend:
                emit_av(*item)

            # finalize: one copy releases the av accumulator immediately (the
            # next block's AV matmuls reuse its banks); reciprocals/mults then
            # work from the SBUF copy, fully overlapped with the next block.
            # Even head: numerator on parts 0:64, denominator on 64:128; odd
            # head mirrored. DVE lanes are partition-locked and
            # reciprocal_approx_fast only works from base partition 0, so the
            # even head's denominator is DMA-moved to lanes 0:64 first; the
            # odd head's reciprocal computes at base 0 and is DMA'd up.
            cpy = rpool.tile([128, 2, 512], F32, tag="cpy")
            nc.scalar.copy(out=cpy, in_=av)
            if DEBUG_TAPS and cc == 0 and iq == 0:
                nc.sync.dma_start(out=aps["dbg_av"], in_=cpy)
            dsw = rpool.tile([64, 512], F32, tag="dsw")
            nc.sync.dma_start(out=dsw[:, :], in_=cpy[64:128, 0, :])
            re = rpool.tile([64, 512], F32, tag="re")
            nc.vector.reciprocal_approx_fast(out=re[:, :], in_=dsw[:, :])
            ro = rpool.tile([64, 512], F32, tag="ro")
            nc.vector.reciprocal_approx_fast(out=ro[:, :], in_=cpy[0:64, 1, :])
            ru = rpool.tile([128, 512], F32, tag="ru")
            nc.sync.dma_start(out=ru[64:128, :], in_=ro[:, :])
            isl = slice(iq * 512, (iq + 1) * 512)
            nc.vector.tensor_tensor(out=aoT[cc][0:64, isl],
                                    in0=cpy[0:64, 0, :], in1=re[:, :], op=OP.mult)
            nc.vector.tensor_tensor(out=aoT[cc][64:128, isl],
                                    in0=cpy[64:128, 1, :], in1=ru[64:128, :], op=OP.mult)

            if cc == 1:
                pending_op.extend((iq, mc) for mc in range(4))

    # tail: whatever out-proj blocks remain (the last i-block, 4 mc)
    for p, mc in pending_op:
        outproj_mc(p, mc)

    if DEBUG_TAPS:
        for g in range(4):
            nc.sync.dma_start(out=aps[f"dbg_xnT{g}"], in_=xnT_g[g])
            nc.sync.dma_start(out=aps[f"dbg_v{g}"], in_=v_g[g])
        for ccx in range(2):
            for p in range(4):
                nc.sync.dma_start(out=aps[f"dbg_qT{ccx}{p}"], in_=qT_t[ccx][p])
                nc.sync.dma_start(out=aps[f"dbg_kT{ccx}{p}"], in_=kT_t[ccx][p])
            nc.sync.dma_start(out=aps[f"dbg_aoT{ccx}"], in_=aoT[ccx])


_CACHE: dict = {}


def _build(affine: bool):
    key = ("nc", affine)
    if key in _CACHE:
        return _CACHE[key]
    nc = bacc.Bacc("TRN2", target_bir_lowering=False, debug=False,
                   num_devices=NCORES)
    aps = {
        "x": nc.dram_tensor("x", [N, DIM], F32, kind="ExternalInput").ap(),
        "pb": nc.dram_tensor("pb", [N], F32, kind="ExternalInput").ap(),
        "wq": nc.dram_tensor("wq", [DIM, 256], BF16, kind="ExternalInput").ap(),
        "wk": nc.dram_tensor("wk", [DIM, 256], BF16, kind="ExternalInput").ap(),
        "wv": nc.dram_tensor("wv", [DIM, 256], BF16, kind="ExternalInput").ap(),
        "wo": nc.dram_tensor("wo", [256, DIM], BF16, kind="ExternalInput").ap(),
        "out": nc.dram_tensor("out", [DIM, N], BF16, kind="ExternalOutput").ap(),
    }
    if affine:
        aps["gam"] = nc.dram_tensor("gam", [DIM], F32, kind="ExternalInput").ap()
        aps["bet"] = nc.dram_tensor("bet", [DIM], F32, kind="ExternalInput").ap()
    if DEBUG_TAPS:
        for g in range(4):
            aps[f"dbg_xnT{g}"] = nc.dram_tensor(f"dbg_xnT{g}", [128, DC, 512], BF16, kind="ExternalOutput").ap()
            aps[f"dbg_v{g}"] = nc.dram_tensor(f"dbg_v{g}", [128, 4, 4, 128], BF16, kind="ExternalOutput").ap()
        for ccx in range(2):
            for p in range(4):
                aps[f"dbg_qT{ccx}{p}"] = nc.dram_tensor(f"dbg_qT{ccx}{p}", [128, 512], BF16, kind="ExternalOutput").ap()
                aps[f"dbg_kT{ccx}{p}"] = nc.dram_tensor(f"dbg_kT{ccx}{p}", [128, 512], BF16, kind="ExternalOutput").ap()
            aps[f"dbg_aoT{ccx}"] = nc.dram_tensor(f"dbg_aoT{ccx}", [128, N], BF16, kind="ExternalOutput").ap()
        aps["dbg_av"] = nc.dram_tensor("dbg_av", [128, 2, 512], F32, kind="ExternalOutput").ap()
        aps["dbg_r"] = nc.dram_tensor("dbg_r", [128, 512], F32, kind="ExternalOutput").ap()
        aps["dbg_rd0"] = nc.dram_tensor("dbg_rd0", [64, 512], F32, kind="ExternalOutput").ap()
        aps["dbg_rd1"] = nc.dram_tensor("dbg_rd1", [64, 512], F32, kind="ExternalOutput").ap()
        aps["dbg_dd"] = nc.dram_tensor("dbg_dd", [128, 2, 512], F32, kind="ExternalOutput").ap()
        aps["dbg_eT"] = nc.dram_tensor("dbg_eT", [128, 2, 512], BF16, kind="ExternalOutput").ap()
    with tile.TileContext(nc) as tc:
        with ExitStack() as ctx:
            _emit(tc, ctx, aps, affine)
    nc.compile()
    _CACHE[key] = nc
    return nc


def _prep_in_maps(x, pose_bias, ln_gamma, ln_beta, w_qkv, w_out, beta):
    x = np.asarray(x, np.float32)
    pose = np.asarray(pose_bias, np.float32)
    gam = np.asarray(ln_gamma, np.float32)
    bet = np.asarray(ln_beta, np.float32)
    wqkv = np.asarray(w_qkv, np.float32)
    wo = np.asarray(w_out, np.float32)
    bval = float(np.asarray(beta))
    affine = not (np.all(gam == 1.0) and np.all(bet == 0.0))
    in_maps = []
    for c in range(NCORES):
        b, g = c // 2, c % 2
        sl = slice(g * 256, (g + 1) * 256)
        m = {
            "x": np.ascontiguousarray(x[b]),
            "pb": np.ascontiguousarray(bval * pose[b]),
            "wq": np.ascontiguousarray(wqkv[:, 0:512][:, sl]).astype(BFNP),
            "wk": np.ascontiguousarray(wqkv[:, 512:1024][:, sl]).astype(BFNP),
            "wv": np.ascontiguousarray(wqkv[:, 1024:1536][:, sl]).astype(BFNP),
            "wo": np.ascontiguousarray(wo[sl, :]).astype(BFNP),
        }
        if affine:
            m["gam"] = gam
            m["bet"] = bet
        in_maps.append(m)
    return in_maps, affine


def _gather(results):
    outs = []
    for b in range(B):
        o = results[2 * b]["out"].astype(np.float32) + results[2 * b + 1]["out"].astype(np.float32)
        outs.append(o.T)
    return np.ascontiguousarray(np.stack(outs))


def _ensure_ntff_shim():
    """This image's antenv lacks axon_hooks; register the NTFF profile hook
    ourselves so run_bass_kernel_spmd(trace=True) can capture exec time."""
    import types
    if "antenv.axon_hooks" in sys.modules:
        return
    mod = types.ModuleType("antenv.axon_hooks")
    state = {"hook": None}
    mod.set_axon_ntff_profile_hook = lambda h: state.__setitem__("hook", h)
    mod.get_axon_ntff_profile_hook = lambda: state["hook"]
    sys.modules["antenv.axon_hooks"] = mod
    try:
        from trn_agent_boot.trn_boot import _ntff_profile_via_ctypes
        mod.set_axon_ntff_profile_hook(
            _ntff_profile_via_ctypes("/opt/axon/libaxon_pjrt.so"))
    except Exception:
        pass


def run(trace=False, **inputs):
    if trace:
        _ensure_ntff_shim()
    in_maps, affine = _prep_in_maps(**inputs)
    nc = _build(affine)
    res = run_bass_kernel_spmd(nc, in_maps, core_ids=list(range(NCORES)),
                               trace=trace)
    return _gather(res.results), res


def kernel(**inputs) -> np.ndarray:
    out, _ = run(trace=False, **inputs)
    return out


# revision 67
# speedup vs baseline: 1.0273x; 1.0273x over previous
"""Trainium2 Bass kernel for nn_Attention_45578192945380.

Full (unsharded) inputs -> full output. Sharding: core c handles batch b=c//2
and head group g=c%2 (heads 4g..4g+4). Zero cross-core communication; the two
cores sharing a batch produce partial out-projections that are summed on host.

Key structure (v2, rewritten from the 229us baseline):
  - softmax_j(s_ij + B*bias_i + B*bias_j) == softmax_j(s_ij + B*bias_j); the
    remaining e^{B*bias_j} factor is folded into the AV weight columns
    (v_j * e^{b_j} and an e^{b_j} "denominator" block), making the exp input
    bias-free so one ACTIVATE can span any PSUM layout.
  - scores computed transposed (sT[j,i] = k_j . q_i); AV lhsT = [v*eb | eb]
    with the eb block 64 wide, so AV rows 64:127 are the softmax denominator
    replicated 64x -> reciprocal+mult directly from PSUM, no broadcasts.
  - xn transpose done by the DMA XBAR (dma_start_transpose), not the PE.
  - attention processes head PAIRS with row-tiled concurrent K=64 dots
    matmuls (tile_position (0,0)/(64,0)) -> ~2x dots throughput.
  - exp is split across engines: ScalarE table exp for head0 (+ fused
    both-head reads late in each block), and a Schraudolph-style fast exp on
    VectorE for head1 (affine in log2 domain -> int16 round -> bits are bf16).
    Calibrated vs HW: convert is round-to-nearest; B tuned for zero-mean
    relative error (max ~4%, cancels in the softmax average).
  - out-projection streams per 512-col i-block as soon as both head-pairs
    finished that block; results DMA PSUM->DRAM in fp32, host sums partials.
"""

import os
import sys
from contextlib import ExitStack

import numpy as np

for _p in ("/opt/trn_rl_repo", "/root/.axon_site/_ro/trn_rl_repo"):
    if os.path.isdir(_p) and _p not in sys.path:
        sys.path.insert(0, _p)

import ml_dtypes

import concourse.bass as bass
import concourse.bacc as bacc
import concourse.tile as tile
from concourse import mybir
from concourse.bass_utils import run_bass_kernel_spmd
from concourse.masks import make_identity

F32 = mybir.dt.float32
BF16 = mybir.dt.bfloat16
I16 = mybir.dt.int16
AF = mybir.ActivationFunctionType
OP = mybir.AluOpType
BFNP = ml_dtypes.bfloat16

B, N, DIM = 4, 2048, 512
HEADS, DH = 8, 64
EPS = 1e-5
NT = N // 128          # 16 n-chunks of 128
DC = DIM // 128        # 4 d-chunks
SCALE = DH ** -0.5     # 0.125
NCORES = 8

# Schraudolph fast-exp in bf16 bits: bf16_bits(e^s) ~= rint(A_SCH*s + B_SCH)
# (A_SCH folds the 1/sqrt(dh) score scale; B_SCH calibrated for zero-mean
# relative error under the HW's round-to-nearest fp32->int16 convert.)
A_SCH = (128.0 / float(np.log(2.0))) * SCALE
B_SCH = 16248.62
DVE_J = 12             # per (pair, iq): J < DVE_J -> head1 exp on VectorE
DEBUG_TAPS = False     # emit intermediate-tensor DMAs for debugging


def _emit(tc: tile.TileContext, ctx: ExitStack, aps: dict, affine: bool):
    nc = tc.nc

    const = ctx.enter_context(tc.tile_pool(name="const", bufs=1))
    big = ctx.enter_context(tc.tile_pool(name="big", bufs=1))

    # ---------------- input DMAs (coalesced) ----------------
    # x in 4 group-issues on the sync HWDGE queue; weights/pb on gpsimd SWDGE.
    # x spread across all three DMA rings: each ring drains its DMAs
    # serially, so putting everything on one ring serializes the prologue
    x_g = [big.tile([128, 4, DIM], F32, name=f"x{g}") for g in range(4)]
    xr = aps["x"].rearrange("(t p) d -> p t d", p=128)
    nc.sync.dma_start(out=x_g[0], in_=xr[:, 0:4, :])
    nc.scalar.dma_start(out=x_g[1], in_=xr[:, 4:8, :])
    nc.sync.dma_start(out=x_g[2], in_=xr[:, 8:12, :])
    nc.scalar.dma_start(out=x_g[3], in_=xr[:, 12:16, :])

    wq_sb = const.tile([128, DC, 256], BF16)
    wk_sb = const.tile([128, DC, 256], BF16)
    wv_sb = const.tile([128, DC, 256], BF16)
    wo_sb = const.tile([128, 2, 512], BF16)
    nc.gpsimd.dma_start(out=wq_sb, in_=aps["wq"].rearrange("(c p) m -> p c m", p=128))
    nc.gpsimd.dma_start(out=wk_sb, in_=aps["wk"].rearrange("(c p) m -> p c m", p=128))
    nc.gpsimd.dma_start(out=wv_sb, in_=aps["wv"].rearrange("(c p) m -> p c m", p=128))
    nc.gpsimd.dma_start(out=wo_sb, in_=aps["wo"].rearrange("(c p) m -> p c m", p=128))
    pb_sb = const.tile([128, NT], F32)
    nc.gpsimd.dma_start(out=pb_sb[:, :], in_=aps["pb"].rearrange("(t p) -> p t", p=128))

    gam_bc = bet_bc = None
    if affine:
        gam_bc = const.tile([128, DIM], F32)
        bet_bc = const.tile([128, DIM], F32)
        ga, be = aps["gam"], aps["bet"]
        nc.sync.dma_start(
            out=gam_bc[:, :],
            in_=bass.AP(tensor=ga.tensor, offset=ga.offset, ap=[[0, 128]] + list(ga.ap)),
        )
        nc.sync.dma_start(
            out=bet_bc[:, :],
            in_=bass.AP(tensor=be.tensor, offset=be.offset, ap=[[0, 128]] + list(be.ap)),
        )

    # ---------------- persistent SBUF ----------------
    xnT_g = [big.tile([128, DC, 512], BF16, name=f"xnT{g}") for g in range(4)]
    qT_t = [[big.tile([128, 512], BF16, name=f"qT{cc}{p}") for p in range(4)]
            for cc in range(2)]
    kT_t = [[big.tile([128, 512], BF16, name=f"kT{cc}{p}") for p in range(4)]
            for cc in range(2)]
    v_g = [big.tile([128, 4, 4, 128], BF16, name=f"v{g}") for g in range(4)]
    aoT = [big.tile([128, N], BF16, name=f"aoT{cc}") for cc in range(2)]
    stats = const.tile([128, NT, 2], F32)
    lnv = const.tile([128, NT], F32)
    rstd = const.tile([128, NT], F32)
    negmr = const.tile([128, NT], F32)
    pbe = const.tile([128, NT], F32)
    eps_sb = const.tile([128, 1], F32)
    nc.vector.memset(eps_sb, EPS)
    zero_sb = const.tile([128, 1], F32)
    nc.vector.memset(zero_sb, 0.0)
    ones256 = const.tile([128, 256], BF16)
    nc.gpsimd.memset(ones256, 1.0)
    ident = const.tile([128, 128], BF16)
    make_identity(nc, ident)

    # pbe = exp(beta*pb) (pb premultiplied by beta on host)
    nc.scalar.activation(out=pbe, in_=pb_sb, func=AF.Exp, bias=zero_sb[:, :])

    # ---------------- PSUM pools ----------------
    # dots: 4 single-bank tiles; av: double-buffered 2-bank accumulators.
    # qk pieces / v chunks / out-proj borrow dots slots.
    dots_pool = ctx.enter_context(tc.tile_pool(name="dots", bufs=6, space="PSUM"))
    av_pool = ctx.enter_context(tc.tile_pool(name="av", bufs=1, space="PSUM"))

    ph1 = ctx.enter_context(tc.tile_pool(name="ph1", bufs=4))
    epool = ctx.enter_context(tc.tile_pool(name="epool", bufs=8))
    rpool = ctx.enter_context(tc.tile_pool(name="rpool", bufs=2))
    spool = ctx.enter_context(tc.tile_pool(name="spool", bufs=4))

    # debug.py relies on this layout note: v_g even heads [v*eb | eb],
    # odd heads [eb | v*eb].

    # ---------------- LN stats ----------------
    def finalize_stats(sl):
        nc.scalar.activation(out=lnv[:, sl], in_=stats[:, sl, 1], func=AF.Ln, bias=eps_sb[:, :])
        nc.scalar.activation(out=rstd[:, sl], in_=lnv[:, sl], func=AF.Exp, bias=zero_sb[:, :], scale=-0.5)
        nc.vector.tensor_scalar(out=stats[:, sl, 0], in0=stats[:, sl, 0], scalar1=-1.0, scalar2=None, op0=OP.mult)
        nc.vector.tensor_tensor_scan(out=negmr[:, sl], data0=stats[:, sl, 0], data1=rstd[:, sl],
                                     initial=0.0, op0=OP.bypass, op1=OP.mult)

    # ---------------- LN apply + XBAR transpose + V + QK ----------------
    def qk_piece(w_sb, dst, cc, p, evac_eng):
        ps = dots_pool.tile([128, 512], F32, tag="d", name=f"qk_{id(dst)}_{cc}_{p}")
        for dc in range(DC):
            nc.tensor.matmul(
                ps, w_sb[:, dc, cc * 128:(cc + 1) * 128],
                xnT_g[p][:, dc, :],
                start=(dc == 0), stop=(dc == DC - 1),
            )
        if evac_eng == "scalar":
            nc.scalar.copy(out=dst, in_=ps)
        else:
            nc.vector.tensor_copy(out=dst, in_=ps)

    def prologue_group(g):
        """bn + LN + transpose + QK(cc0) + V for one 4-chunk group."""
        for tt in range(4):
            t = 4 * g + tt
            st6 = ph1.tile([128, 6], F32, tag="bnst")
            nc.vector.bn_stats(out=st6, in_=x_g[g][:, tt, :])
            nc.vector.bn_aggr(out=stats[:, t, :], in_=st6)
        finalize_stats(slice(4 * g, 4 * g + 4))
        for tt in range(4):
            t = 4 * g + tt
            if affine:
                xh = ph1.tile([128, DIM], F32, tag="xh")
                nc.vector.tensor_scalar(
                    out=xh, in0=x_g[g][:, tt, :],
                    scalar1=rstd[:, t:t + 1], scalar2=negmr[:, t:t + 1],
                    op0=OP.mult, op1=OP.add,
                )
                xg = ph1.tile([128, DIM], F32, tag="xg")
                nc.vector.tensor_tensor(out=xg, in0=xh, in1=gam_bc[:, :], op=OP.mult)
                xn_t = ph1.tile([128, DIM], BF16, tag="xn")
                nc.vector.tensor_tensor(out=xn_t, in0=xg, in1=bet_bc[:, :], op=OP.add)
            else:
                xn_t = ph1.tile([128, DIM], BF16, tag="xn")
                nc.vector.tensor_scalar(
                    out=xn_t, in0=x_g[g][:, tt, :],
                    scalar1=rstd[:, t:t + 1], scalar2=negmr[:, t:t + 1],
                    op0=OP.mult, op1=OP.add,
                )
            if g < 2:
                # groups 0/1: PE transpose (xn.T @ I) — the DMA rings are
                # still busy landing x, and this warms the HAM clock gate
                tp = dots_pool.tile([128, 512], F32, tag="d", name=f"tp_{t}")
                for dc in range(DC):
                    nc.tensor.matmul(tp[:, dc * 128:(dc + 1) * 128],
                                     xn_t[:, dc * 128:(dc + 1) * 128],
                                     ident, start=True, stop=True)
                nc.scalar.copy(
                    out=xnT_g[g][:, :, tt * 128:(tt + 1) * 128],
                    in_=tp.rearrange("p (dc j) -> p dc j", dc=DC))
            else:
                # groups 2/3: XBAR transpose, rings are free by now
                # out[p, dc, j] = xn[j, dc*128+p]
                teng = nc.sync if tt % 2 == 0 else nc.scalar
                teng.dma_start_transpose(
                    out=xnT_g[g][:, :, tt * 128:(tt + 1) * 128], in_=xn_t)
        # QK pieces (cc0 only; cc1 is injected into later attention blocks)
        qk_piece(wq_sb, qT_t[0][g], 0, g, "scalar")
        qk_piece(wk_sb, kT_t[0][g], 0, g, "scalar")
        # V chunks: even heads [v*eb | eb], odd heads [eb | v*eb] — each
        # head's AV numerator lands on the partition half its aoT rows live
        # on, the denominator on the other (DVE lanes are partition-locked;
        # the reciprocal is DMA-moved across at finalize).
        for tt in range(4):
            t = 4 * g + tt
            ps = dots_pool.tile([128, 512], F32, tag="d", name=f"vps_{t}")
            for dc in range(DC):
                nc.tensor.matmul(
                    ps[:, 0:256], xnT_g[g][:, dc, tt * 128:(tt + 1) * 128],
                    wv_sb[:, dc, :],
                    start=(dc == 0), stop=(dc == DC - 1),
                )
            va = v_g[g][:, tt, :, :]
            v_dst = bass.AP(tensor=va.tensor, offset=va.offset,
                            ap=[list(va.ap[0]), [256, 2], [192, 2], [1, 64]])
            vin = ps[:, 0:256].rearrange("p (a b x) -> p a b x", a=2, b=2)
            nc.scalar.activation(out=v_dst, in_=vin, func=AF.Copy,
                                 bias=0.0, scale=pbe[:, t:t + 1])
            e_dst = bass.AP(tensor=va.tensor, offset=va.offset + 64,
                            ap=[list(va.ap[0]), [256, 2], [64, 2], [1, 64]])
            nc.gpsimd.tensor_scalar(
                out=e_dst,
                in0=ones256.rearrange("p (a b x) -> p a b x", a=2, b=2),
                scalar1=pbe[:, t:t + 1], scalar2=None, op0=OP.mult,
            )

    prologue_group(0)

    # ---------------- attention: head pairs ----------------
    def outproj_mc(p, mc):
        po = dots_pool.tile([128, 512], F32, tag="d", name=f"po_{p}_{mc}")
        nc.tensor.matmul(po, wo_sb[:, 0, mc * 128:(mc + 1) * 128],
                         aoT[0][:, p * 512:(p + 1) * 512], start=True, stop=False)
        nc.tensor.matmul(po, wo_sb[:, 1, mc * 128:(mc + 1) * 128],
                         aoT[1][:, p * 512:(p + 1) * 512], start=False, stop=True)
        st = spool.tile([128, 512], BF16, tag="st")
        if mc % 2 == 0:
            nc.scalar.copy(out=st, in_=po)
        else:
            nc.vector.tensor_copy(out=st, in_=po)
        nc.gpsimd.dma_start(
            out=aps["out"][mc * 128:(mc + 1) * 128, p * 512:(p + 1) * 512],
            in_=st,
        )

    pending_op = []  # out-proj i-blocks deferred into the next block's J-loop
    for cc in range(2):
        h0, h1 = 2 * cc, 2 * cc + 1
        for iq in range(4):
            # remaining prologue groups pipeline into the first block;
            # cc=1 QK pieces are injected near the end of each cc=0 block
            hooks = {}
            if cc == 0 and iq == 0:
                hooks = {2: lambda: prologue_group(1),
                         5: lambda: prologue_group(2),
                         9: lambda: prologue_group(3)}
            if cc == 0:
                hooks[13] = lambda p=iq: qk_piece(wq_sb, qT_t[1][p], 1, p, "scalar")
                hooks[14] = lambda p=iq: qk_piece(wk_sb, kT_t[1][p], 1, p, "scalar")
            av = av_pool.tile([128, 2, 512], F32, tag="av", name=f"av_{cc}_{iq}")
            pend = []

            def emit_av(Jp, e0p, e1p):
                g, tt = Jp // 4, Jp % 4
                nc.tensor.matmul(av[:, 0, :], v_g[g][:, tt, h0, :], e0p,
                                 start=(Jp == 0), stop=(Jp == NT - 1))
                nc.tensor.matmul(av[:, 1, :], v_g[g][:, tt, h1, :], e1p,
                                 start=(Jp == 0), stop=(Jp == NT - 1))

            for J in range(NT):
                if J in hooks:
                    hooks[J]()
                d0 = dots_pool.tile([128, 512], F32, tag="d", name=f"d0_{cc}_{iq}_{J}")
                d1 = dots_pool.tile([128, 512], F32, tag="d", name=f"d1_{cc}_{iq}_{J}")
                kp = kT_t[cc][J // 4]
                ksl = slice((J % 4) * 128, (J % 4) * 128 + 128)
                qp = qT_t[cc][iq]
                nc.tensor.matmul(d0, kp[0:64, ksl], qp[0:64, :],
                                 start=True, stop=True)
                nc.tensor.matmul(d1, kp[64:128, ksl], qp[64:128, :],
                                 start=True, stop=True)
                # separate eT tiles per head: ACT and DVE each own one, so the
                # two exp engines never serialize on a shared writer tile
                e0 = epool.tile([128, 512], BF16, tag="e0")
                e1 = epool.tile([128, 512], BF16, tag="e1")
                nc.scalar.activation(out=e0, in_=d0,
                                     func=AF.Exp, bias=zero_sb[:, :], scale=SCALE)
                if J < DVE_J:
                    nc.vector.tensor_scalar(
                        out=e1.bitcast(I16), in0=d1,
                        scalar1=A_SCH, scalar2=B_SCH, op0=OP.mult, op1=OP.add,
                    )
                else:
                    nc.scalar.activation(out=e1, in_=d1,
                                         func=AF.Exp, bias=zero_sb[:, :], scale=SCALE)
                if len(pend) >= 2:
                    emit_av(*pend.pop(0))
                pend.append((J, e0, e1))
                # spread a deferred out-proj column block into this J-stream
                if pending_op and J in (6, 8, 10, 12):
                    outproj_mc(*pending_op.pop(0))
            for item in pend:
                emit_av(*item)

            # finalize: one copy releases the av accumulator immediately (the
            # next block's AV matmuls reuse its banks); reciprocals/mults then
            # work from the SBUF copy, fully overlapped with the next block.
            # Even head: numerator on parts 0:64, denominator on 64:128; odd
            # head mirrored. DVE lanes are partition-locked and
            # reciprocal_approx_fast only works from base partition 0, so the
            # even head's denominator is DMA-moved to lanes 0:64 first; the
            # odd head's reciprocal computes at base 0 and is DMA'd up.
            cpy = rpool.tile([128, 2, 512], F32, tag="cpy")
            nc.scalar.copy(out=cpy, in_=av)
            if DEBUG_TAPS and cc == 0 and iq == 0:
                nc.sync.dma_start(out=aps["dbg_av"], in_=cpy)
            dsw = rpool.tile([64, 512], F32, tag="dsw")
            nc.sync.dma_start(out=dsw[:, :], in_=cpy[64:128, 0, :])
            re = rpool.tile([64, 512], F32, tag="re")
            nc.vector.reciprocal_approx_fast(out=re[:, :], in_=dsw[:, :])
            ro = rpool.tile([64, 512], F32, tag="ro")
            nc.vector.reciprocal_approx_fast(out=ro[:, :], in_=cpy[0:64, 1, :])
            ru = rpool.tile([128, 512], F32, tag="ru")
            nc.sync.dma_start(out=ru[64:128, :], in_=ro[:, :])
            isl = slice(iq * 512, (iq + 1) * 512)
            nc.vector.tensor_tensor(out=aoT[cc][0:64, isl],
                                    in0=cpy[0:64, 0, :], in1=re[:, :], op=OP.mult)
            nc.vector.tensor_tensor(out=aoT[cc][64:128, isl],
                                    in0=cpy[64:128, 1, :], in1=ru[64:128, :], op=OP.mult)

            if cc == 1:
                pending_op.extend((iq, mc) for mc in range(4))

    # tail: whatever out-proj blocks remain (the last i-block, 4 mc)
    for p, mc in pending_op:
        outproj_mc(p, mc)

    if DEBUG_TAPS:
        for g in range(4):
            nc.sync.dma_start(out=aps[f"dbg_xnT{g}"], in_=xnT_g[g])
            nc.sync.dma_start(out=aps[f"dbg_v{g}"], in_=v_g[g])
        for ccx in range(2):
            for p in range(4):
                nc.sync.dma_start(out=aps[f"dbg_qT{ccx}{p}"], in_=qT_t[ccx][p])
                nc.sync.dma_start(out=aps[f"dbg_kT{ccx}{p}"], in_=kT_t[ccx][p])
            nc.sync.dma_start(out=aps[f"dbg_aoT{ccx}"], in_=aoT[ccx])


_CACHE: dict = {}


def _build(affine: bool):
    key = ("nc", affine)
    if key in _CACHE:
        return _CACHE[key]
    nc = bacc.Bacc("TRN2", target_bir_lowering=False, debug=False,
                   num_devices=NCORES)
    aps = {
        "x": nc.dram_tensor("x", [N, DIM], F32, kind="ExternalInput").ap(),
        "pb": nc.dram_tensor("pb", [N], F32, kind="ExternalInput").ap(),
        "wq": nc.dram_tensor("wq", [DIM, 256], BF16, kind="ExternalInput").ap(),
        "wk": nc.dram_tensor("wk", [DIM, 256], BF16, kind="ExternalInput").ap(),
        "wv": nc.dram_tensor("wv", [DIM, 256], BF16, kind="ExternalInput").ap(),
        "wo": nc.dram_tensor("wo", [256, DIM], BF16, kind="ExternalInput").ap(),
        "out": nc.dram_tensor("out", [DIM, N], BF16, kind="ExternalOutput").ap(),
    }
    if affine:
        aps["gam"] = nc.dram_tensor("gam", [DIM], F32, kind="ExternalInput").ap()
        aps["bet"] = nc.dram_tensor("bet", [DIM], F32, kind="ExternalInput").ap()
    if DEBUG_TAPS:
        for g in range(4):
            aps[f"dbg_xnT{g}"] = nc.dram_tensor(f"dbg_xnT{g}", [128, DC, 512], BF16, kind="ExternalOutput").ap()
            aps[f"dbg_v{g}"] = nc.dram_tensor(f"dbg_v{g}", [128, 4, 4, 128], BF16, kind="ExternalOutput").ap()
        for ccx in range(2):
            for p in range(4):
                aps[f"dbg_qT{ccx}{p}"] = nc.dram_tensor(f"dbg_qT{ccx}{p}", [128, 512], BF16, kind="ExternalOutput").ap()
                aps[f"dbg_kT{ccx}{p}"] = nc.dram_tensor(f"dbg_kT{ccx}{p}", [128, 512], BF16, kind="ExternalOutput").ap()
            aps[f"dbg_aoT{ccx}"] = nc.dram_tensor(f"dbg_aoT{ccx}", [128, N], BF16, kind="ExternalOutput").ap()
        aps["dbg_av"] = nc.dram_tensor("dbg_av", [128, 2, 512], F32, kind="ExternalOutput").ap()
        aps["dbg_r"] = nc.dram_tensor("dbg_r", [128, 512], F32, kind="ExternalOutput").ap()
        aps["dbg_rd0"] = nc.dram_tensor("dbg_rd0", [64, 512], F32, kind="ExternalOutput").ap()
        aps["dbg_rd1"] = nc.dram_tensor("dbg_rd1", [64, 512], F32, kind="ExternalOutput").ap()
        aps["dbg_dd"] = nc.dram_tensor("dbg_dd", [128, 2, 512], F32, kind="ExternalOutput").ap()
        aps["dbg_eT"] = nc.dram_tensor("dbg_eT", [128, 2, 512], BF16, kind="ExternalOutput").ap()
    with tile.TileContext(nc) as tc:
        with ExitStack() as ctx:
            _emit(tc, ctx, aps, affine)
    nc.compile()
    _CACHE[key] = nc
    return nc


def _prep_in_maps(x, pose_bias, ln_gamma, ln_beta, w_qkv, w_out, beta):
    x = np.asarray(x, np.float32)
    pose = np.asarray(pose_bias, np.float32)
    gam = np.asarray(ln_gamma, np.float32)
    bet = np.asarray(ln_beta, np.float32)
    wqkv = np.asarray(w_qkv, np.float32)
    wo = np.asarray(w_out, np.float32)
    bval = float(np.asarray(beta))
    affine = not (np.all(gam == 1.0) and np.all(bet == 0.0))
    in_maps = []
    for c in range(NCORES):
        b, g = c // 2, c % 2
        sl = slice(g * 256, (g + 1) * 256)
        m = {
            "x": np.ascontiguousarray(x[b]),
            "pb": np.ascontiguousarray(bval * pose[b]),
            "wq": np.ascontiguousarray(wqkv[:, 0:512][:, sl]).astype(BFNP),
            "wk": np.ascontiguousarray(wqkv[:, 512:1024][:, sl]).astype(BFNP),
            "wv": np.ascontiguousarray(wqkv[:, 1024:1536][:, sl]).astype(BFNP),
            "wo": np.ascontiguousarray(wo[sl, :]).astype(BFNP),
        }
        if affine:
            m["gam"] = gam
            m["bet"] = bet
        in_maps.append(m)
    return in_maps, affine


def _gather(results):
    outs = []
    for b in range(B):
        o = results[2 * b]["out"].astype(np.float32) + results[2 * b + 1]["out"].astype(np.float32)
        outs.append(o.T)
    return np.ascontiguousarray(np.stack(outs))


def _ensure_ntff_shim():
    """This image's antenv lacks axon_hooks; register the NTFF profile hook
    ourselves so run_bass_kernel_spmd(trace=True) can capture exec time."""
    import types
    if "antenv.axon_hooks" in sys.modules:
        return
    mod = types.ModuleType("antenv.axon_hooks")
    state = {"hook": None}
    mod.set_axon_ntff_profile_hook = lambda h: state.__setitem__("hook", h)
    mod.get_axon_ntff_profile_hook = lambda: state["hook"]
    sys.modules["antenv.axon_hooks"] = mod
    try:
        from trn_agent_boot.trn_boot import _ntff_profile_via_ctypes
        mod.set_axon_ntff_profile_hook(
            _ntff_profile_via_ctypes("/opt/axon/libaxon_pjrt.so"))
    except Exception:
        pass


def run(trace=False, **inputs):
    if trace:
        _ensure_ntff_shim()
    in_maps, affine = _prep_in_maps(**inputs)
    nc = _build(affine)
    res = run_bass_kernel_spmd(nc, in_maps, core_ids=list(range(NCORES)),
                               trace=trace)
    return _gather(res.results), res


def kernel(**inputs) -> np.ndarray:
    out, _ = run(trace=False, **inputs)
    return out
